# revision 2
# baseline (speedup 1.0000x reference)
"""Causal self-attention kernel for Trainium2 (8 NeuronCores, data-parallel).

Problem: B=8, T=2048, C=1024 single-head causal attention:
    qkv = x @ w_attn + b_attn ; q,k,v = split(qkv)
    attn = softmax(q @ k.T / sqrt(C) + causal_mask)
    out  = (attn @ v) @ w_proj + b_proj

Sharding: pure data parallel — one batch element per core, weights replicated,
no collectives.

Per-core algorithm (all matmuls bf16 operands, fp32 PSUM accumulate):
  host: xT = x[b].T cast bf16 (so the contraction dim is the partition dim
        everywhere on device; no on-device transposes needed anywhere).
  ph1:  qT[e,t], kT[e,s]  <- matmul(lhsT=w_qk[c,e-tile], rhs=xT[c,t])   [e,t] layout
        v[t,c']           <- matmul(lhsT=xT[c,t-tile],  rhs=w_v[c,c'])  natural layout
  ph2:  per 512-wide t-chunk ("supertile"), per 128-wide s-tile (causal only):
        ST[s,t]  <- matmul(lhsT=kT[e,s-tile], rhs=qT[e,t-chunk])  (8 e-tiles acc)
        P~T[s,t] <- exp(ST/sqrt(C) + mask)   (no max-subtract; logits are O(1))
        sums[t]  <- matmul(lhsT=ones[s,1], rhs=P~T)  (acc over s-tiles)
        OT[c',t] <- matmul(lhsT=v[s-tile,c'-tile], rhs=P~T[s-tile,t-chunk])
        out[t,d] <- matmul(lhsT=OT[c',t-tile], rhs=w_proj[c',d]) * (1/sums[t])
  The 1/sums normalization is folded into the final PSUM->SBUF copy as a
  per-partition activation scale (everything between exp and out is linear).

b_attn is folded in by augmenting x with a ones column (padded to a full
128-partition tile) only when it is nonzero; b_proj is added on the host.
"""

import sys

if "/opt/trn_rl_repo" not in sys.path:
    sys.path.insert(0, "/opt/trn_rl_repo")

import numpy as np
import ml_dtypes

import concourse.bacc as bacc
import concourse.mybir as mybir
import concourse.tile as tile
from concourse.bass_utils import run_bass_kernel_spmd

B, T, C = 8, 2048, 1024
P = 128  # partitions
TCH = 512  # t-chunk (moving free dim)
N_TT = T // P  # 16 t-tiles
N_SUP = T // TCH  # 4 supertiles
N_ET = C // P  # 8 e-tiles (q/k feature dim)
SCALE = 1.0 / float(np.sqrt(np.float32(C)))
NEG = -10000000000.0

BF16 = mybir.dt.bfloat16
FP32 = mybir.dt.float32

_cache = {}


def _build(n_ct):
    """Build the SPMD Bass program. n_ct = number of 128-wide c-tiles of the
    (possibly ones-augmented) input feature dim."""
    Caug = n_ct * P
    nc = bacc.Bacc("TRN2", target_bir_lowering=False, debug=False, num_devices=8)

    xT_d = nc.dram_tensor("xT", [Caug, T], BF16, kind="ExternalInput").ap()
    wqk_d = nc.dram_tensor("wqk", [Caug, 2 * C], BF16, kind="ExternalInput").ap()
    wv_d = nc.dram_tensor("wv", [Caug, C], BF16, kind="ExternalInput").ap()
    wp_d = nc.dram_tensor("wp", [C, C], BF16, kind="ExternalInput").ap()
    maskT_d = nc.dram_tensor("maskT", [P, P], FP32, kind="ExternalInput").ap()
    out_d = nc.dram_tensor("out", [T, C], FP32, kind="ExternalOutput").ap()
    scr_d = nc.dram_tensor("scr", [N_SUP, TCH], FP32, kind="ExternalOutput").ap()

    with tile.TileContext(nc) as tc:
        with (
            tc.tile_pool(name="persist", bufs=1) as persist,
            tc.tile_pool(name="small", bufs=1) as small,
        ):
            # persistent SBUF arrays
            qT = [persist.tile([P, T], BF16, name=f"qT{e}", tag=f"qT{e}") for e in range(N_ET)]
            kT = [persist.tile([P, T], BF16, name=f"kT{e}", tag=f"kT{e}") for e in range(N_ET)]
            v = [persist.tile([P, C], BF16, name=f"v{t}", tag=f"v{t}") for t in range(N_TT)]
            maskT = small.tile([P, P], FP32, name="maskT", tag="maskT")
            ones = small.tile([P, 1], BF16, name="ones", tag="ones")
            nc.sync.dma_start(maskT[:], maskT_d[:])
            nc.vector.memset(ones[:], 1.0)

            # ---------------- phase 1: projections ----------------
            with (
                tc.tile_pool(name="ph1", bufs=1) as ph1,
                tc.tile_pool(name="ph1ps", bufs=4, space="PSUM") as ph1ps,
            ):
                xT = [ph1.tile([P, T], BF16, name=f"xT{c}", tag=f"xT{c}") for c in range(n_ct)]
                wqk = [ph1.tile([P, 2 * C], BF16, name=f"wqk{c}", tag=f"wqk{c}") for c in range(n_ct)]
                wv = [ph1.tile([P, C], BF16, name=f"wv{c}", tag=f"wv{c}") for c in range(n_ct)]
                for c in range(n_ct):
                    nc.sync.dma_start(xT[c][:], xT_d[c * P : (c + 1) * P, :])
                    nc.sync.dma_start(wqk[c][:], wqk_d[c * P : (c + 1) * P, :])
                    nc.sync.dma_start(wv[c][:], wv_d[c * P : (c + 1) * P, :])

                # qT/kT: psum[e-tile, t-chunk] = sum_c w_qk[c, e].T @ xT[c, t]
                for e in range(2 * N_ET):
                    dst = qT[e] if e < N_ET else kT[e - N_ET]
                    for tc_i in range(T // TCH):
                        ps = ph1ps.tile([P, TCH], FP32, name="qkps", tag="qkps")
                        for c in range(n_ct):
                            nc.tensor.matmul(
                                ps[:],
                                wqk[c][:, e * P : (e + 1) * P],
                                xT[c][:, tc_i * TCH : (tc_i + 1) * TCH],
                                start=(c == 0),
                                stop=(c == n_ct - 1),
                            )
                        eng = nc.vector if (e * 4 + tc_i) % 2 == 0 else nc.scalar
                        if eng is nc.vector:
                            eng.tensor_copy(dst[:, tc_i * TCH : (tc_i + 1) * TCH], ps[:])
                        else:
                            eng.copy(dst[:, tc_i * TCH : (tc_i + 1) * TCH], ps[:])

                # v: psum[t-tile, c'-chunk] = sum_c xT[c, t].T @ w_v[c, c']
                for t in range(N_TT):
                    for cc in range(C // TCH):
                        ps = ph1ps.tile([P, TCH], FP32, name="vps", tag="qkps")
                        for c in range(n_ct):
                            nc.tensor.matmul(
                                ps[:],
                                xT[c][:, t * P : (t + 1) * P],
                                wv[c][:, cc * TCH : (cc + 1) * TCH],
                                start=(c == 0),
                                stop=(c == n_ct - 1),
                            )
                        eng = nc.vector if (t * 2 + cc) % 2 == 0 else nc.scalar
                        if eng is nc.vector:
                            eng.tensor_copy(v[t][:, cc * TCH : (cc + 1) * TCH], ps[:])
                        else:
                            eng.copy(v[t][:, cc * TCH : (cc + 1) * TCH], ps[:])

            # ---------------- phase 2: attention + proj ----------------
            with (
                tc.tile_pool(name="ph2", bufs=1) as ph2,
                tc.tile_pool(name="pt_pool", bufs=18) as pt_pool,
                tc.tile_pool(name="ot_pool", bufs=3) as ot_pool,
                tc.tile_pool(name="stage", bufs=3) as stage,
                tc.tile_pool(name="st_ps", bufs=2, space="PSUM") as st_ps,
                tc.tile_pool(name="sums_ps", bufs=1, space="PSUM") as sums_ps,
                tc.tile_pool(name="ot_ps", bufs=2, space="PSUM") as ot_ps,
                tc.tile_pool(name="pr_ps", bufs=2, space="PSUM") as pr_ps,
            ):
                wp = [ph2.tile([P, C], BF16, name=f"wp{c}", tag=f"wp{c}") for c in range(N_ET)]
                for c in range(N_ET):
                    nc.sync.dma_start(wp[c][:], wp_d[c * P : (c + 1) * P, :])

                for i in range(N_SUP):  # supertile: t in [i*TCH, (i+1)*TCH)
                    t0 = i * TCH
                    n_st = 4 * i + 4  # causal s-tiles
                    ptiles = []
                    # --- ST + exp per s-tile ---
                    for j in range(n_st):
                        off = max(0, (j - 4 * i)) * P  # first valid t column
                        st = st_ps.tile([P, TCH], FP32, name="st", tag="st")
                        for e in range(N_ET):
                            nc.tensor.matmul(
                                st[:, off:TCH],
                                kT[e][:, j * P : (j + 1) * P],
                                qT[e][:, t0 + off : t0 + TCH],
                                start=(e == 0),
                                stop=(e == N_ET - 1),
                            )
                        if j >= 4 * i:  # diagonal block: strict-upper (s>t) mask
                            nc.vector.tensor_add(
                                st[:, off : off + P], st[:, off : off + P], maskT[:]
                            )
                        pt = pt_pool.tile([P, TCH], BF16, name="pt", tag="pt")
                        if off:
                            nc.gpsimd.memset(pt[:, :off], 0.0)
                        nc.scalar.activation(
                            pt[:, off:TCH],
                            st[:, off:TCH],
                            mybir.ActivationFunctionType.Exp,
                            scale=SCALE,
                        )
                        ptiles.append(pt)

                    # --- row sums via ones-matmul (acc over s-tiles) ---
                    sums = sums_ps.tile([1, TCH], FP32, name="sums", tag="sums")
                    for j in range(n_st):
                        nc.tensor.matmul(
                            sums[:],
                            ones[:],
                            ptiles[j][:],
                            start=(j == 0),
                            stop=(j == n_st - 1),
                        )
                    rec = stage.tile([1, TCH], FP32, name="rec", tag="rec")
                    nc.vector.reciprocal(rec[:], sums[:])
                    nc.sync.dma_start(scr_d[i : i + 1, :], rec[:])
                    rt = stage.tile([P, N_SUP], FP32, name="rt", tag="rt")
                    nc.sync.dma_start(
                        rt[:], scr_d[i : i + 1, :].rearrange("p (f q) -> (p q) f", q=P)
                    )

                    # --- OT[c'-tile, t-chunk] = sum_s v[s,c'].T @ P~T[s,t] ---
                    ot_sb = []
                    for g in range(N_ET):
                        ot = ot_ps.tile([P, TCH], FP32, name="ot", tag="ot")
                        for j in range(n_st):
                            nc.tensor.matmul(
                                ot[:],
                                v[j][:, g * P : (g + 1) * P],
                                ptiles[j][:],
                                start=(j == 0),
                                stop=(j == n_st - 1),
                            )
                        osb = ot_pool.tile([P, TCH], BF16, name="osb", tag=f"osb{g % 3}")
                        nc.vector.tensor_copy(osb[:], ot[:])
                        ot_sb.append(osb)

                    # --- proj + normalize ---
                    for k in range(TCH // P):  # t-tile within supertile
                        for d in range(C // TCH):
                            pr = pr_ps.tile([P, TCH], FP32, name="pr", tag="pr")
                            for g in range(N_ET):
                                nc.tensor.matmul(
                                    pr[:],
                                    ot_sb[g][:, k * P : (k + 1) * P],
                                    wp[g][:, d * TCH : (d + 1) * TCH],
                                    start=(g == 0),
                                    stop=(g == N_ET - 1),
                                )
                            osb_out = stage.tile([P, TCH], FP32, name="osb_out", tag="osb_out")
                            nc.scalar.activation(
                                osb_out[:],
                                pr[:],
                                mybir.ActivationFunctionType.Copy,
                                scale=rt[:, k : k + 1],
                            )
                            nc.sync.dma_start(
                                out_d[
                                    t0 + k * P : t0 + (k + 1) * P,
                                    d * TCH : (d + 1) * TCH,
                                ],
                                osb_out[:],
                            )

    nc.compile()
    return nc


def kernel(x, w_attn, b_attn, w_proj, b_proj):
    x = np.asarray(x, dtype=np.float32)
    w_attn = np.asarray(w_attn, dtype=np.float32)
    b_attn = np.asarray(b_attn, dtype=np.float32)
    w_proj = np.asarray(w_proj, dtype=np.float32)
    b_proj = np.asarray(b_proj, dtype=np.float32)
    assert x.shape == (B, T, C)

    aug = bool(np.any(b_attn != 0.0))
    n_ct = C // P + (1 if aug else 0)
    key = n_ct
    if key not in _cache:
        _cache[key] = _build(n_ct)
    nc = _cache[key]

    bf = ml_dtypes.bfloat16
    if aug:
        wqk = np.zeros((n_ct * P, 2 * C), dtype=bf)
        wqk[:C] = w_attn[:, : 2 * C].astype(bf)
        wqk[C] = b_attn[: 2 * C].astype(bf)
        wv = np.zeros((n_ct * P, C), dtype=bf)
        wv[:C] = w_attn[:, 2 * C :].astype(bf)
        wv[C] = b_attn[2 * C :].astype(bf)
    else:
        wqk = np.ascontiguousarray(w_attn[:, : 2 * C]).astype(bf)
        wv = np.ascontiguousarray(w_attn[:, 2 * C :]).astype(bf)
    wp = w_proj.astype(bf)

    # strict upper triangle (s > t) additive mask for transposed [s, t] blocks
    maskT = np.where(
        np.arange(P)[:, None] > np.arange(P)[None, :], np.float32(NEG), np.float32(0.0)
    ).astype(np.float32)

    in_maps = []
    for b in range(B):
        xT = np.ascontiguousarray(x[b].T).astype(bf)
        if aug:
            xTa = np.zeros((n_ct * P, T), dtype=bf)
            xTa[:C] = xT
            xTa[C] = bf(1.0)
            xT = xTa
        in_maps.append(
            {"xT": xT, "wqk": wqk, "wv": wv, "wp": wp, "maskT": maskT}
        )

    global _last_in_maps
    _last_in_maps = in_maps
    res = run_bass_kernel_spmd(nc, in_maps, core_ids=list(range(8)))
    out = np.stack([res.results[b]["out"] for b in range(B)]).astype(np.float32)
    if np.any(b_proj != 0.0):
        out = out + b_proj[None, None, :]
    return out


if __name__ == "__main__":
    rng = np.random.default_rng(0)
    x = rng.standard_normal((B, T, C), dtype=np.float32)
    w_attn = rng.standard_normal((C, 3 * C), dtype=np.float32) / np.sqrt(C)
    b_attn = np.zeros(3 * C, dtype=np.float32)
    w_proj = rng.standard_normal((C, C), dtype=np.float32) / np.sqrt(C)
    b_proj = np.zeros(C, dtype=np.float32)
    out = kernel(x, w_attn, b_attn, w_proj, b_proj)
    print(out.shape, out.dtype)


# revision 3
# speedup vs baseline: 1.2074x; 1.2074x over previous
"""Causal self-attention kernel for Trainium2 (8 NeuronCores, data-parallel).

Problem: B=8, T=2048, C=1024 single-head causal attention:
    qkv = x @ w_attn + b_attn ; q,k,v = split(qkv)
    attn = softmax(q @ k.T / sqrt(C) + causal_mask)
    out  = (attn @ v) @ w_proj + b_proj

Sharding: pure data parallel — one batch element per core, weights replicated,
no collectives.

Per-core algorithm (all matmuls bf16 operands, fp32 PSUM accumulate):
  host: xT = x[b].T cast bf16 (so the contraction dim is the partition dim
        everywhere on device; no on-device transposes needed anywhere).
  ph1:  qT[e,t], kT[e,s]  <- matmul(lhsT=w_qk[c,e-tile], rhs=xT[c,t])   [e,t] layout
        v[t,c']           <- matmul(lhsT=xT[c,t-tile],  rhs=w_v[c,c'])  natural layout
  ph2:  per 512-wide t-chunk ("supertile"), per 128-wide s-tile (causal only):
        ST[s,t]  <- matmul(lhsT=kT[e,s-tile], rhs=qT[e,t-chunk])  (8 e-tiles acc)
        P~T[s,t] <- exp(ST/sqrt(C) + mask)   (no max-subtract; logits are O(1))
        sums[t]  <- matmul(lhsT=ones[s,1], rhs=P~T)  (acc over s-tiles)
        OT[c',t] <- matmul(lhsT=v[s-tile,c'-tile], rhs=P~T[s-tile,t-chunk])
        out[t,d] <- matmul(lhsT=OT[c',t-tile], rhs=w_proj[c',d]) * (1/sums[t])
  The 1/sums normalization is folded into the final PSUM->SBUF copy as a
  per-partition activation scale (everything between exp and out is linear).
  The proj stage runs one supertile behind (software pipeline) so the
  sums->reciprocal DMA roundtrip never stalls the PE.

Inner loops are ordered so the matmul stationary operand is reused across
consecutive matmuls (amortizes LDWEIGHTS), and a dummy-matmul warmup stream at
t=0 lifts the PE HAM clock gate to 2.4 GHz while the first DMAs land.

b_attn is folded in by augmenting x with a ones column (padded to a full
128-partition tile) only when it is nonzero; b_proj is added on the host.
"""

import sys

if "/opt/trn_rl_repo" not in sys.path:
    sys.path.insert(0, "/opt/trn_rl_repo")

import numpy as np
import ml_dtypes

import concourse.bacc as bacc
import concourse.mybir as mybir
import concourse.tile as tile
from concourse.bass_utils import run_bass_kernel_spmd

B, T, C = 8, 2048, 1024
P = 128  # partitions
TCH = 512  # t-chunk (moving free dim)
N_TT = T // P  # 16 t-tiles
N_SUP = T // TCH  # 4 supertiles
N_ET = C // P  # 8 e-tiles (q/k feature dim)
SCALE = 1.0 / float(np.sqrt(np.float32(C)))
NEG = -10000000000.0

BF16 = mybir.dt.bfloat16
FP32 = mybir.dt.float32

_cache = {}


def _build(n_ct):
    """Build the SPMD Bass program. n_ct = number of 128-wide c-tiles of the
    (possibly ones-augmented) input feature dim."""
    nc = bacc.Bacc("TRN2", target_bir_lowering=False, debug=False, num_devices=8)

    xT_d = nc.dram_tensor("xT", [n_ct * P, T], BF16, kind="ExternalInput").ap()
    wqk_d = nc.dram_tensor("wqk", [n_ct * P, 2 * C], BF16, kind="ExternalInput").ap()
    wv_d = nc.dram_tensor("wv", [n_ct * P, C], BF16, kind="ExternalInput").ap()
    wp_d = nc.dram_tensor("wp", [C, C], BF16, kind="ExternalInput").ap()
    maskT_d = nc.dram_tensor("maskT", [P, P], FP32, kind="ExternalInput").ap()
    out_d = nc.dram_tensor("out", [T, C], FP32, kind="ExternalOutput").ap()
    scr_d = nc.dram_tensor("scr", [N_SUP, TCH], FP32, kind="ExternalOutput").ap()

    with tile.TileContext(nc) as tc:
        with (
            tc.tile_pool(name="persist", bufs=1) as persist,
            tc.tile_pool(name="small", bufs=1) as small,
            tc.tile_pool(name="warm_ps", bufs=1, space="PSUM") as warm_ps,
        ):
            # PE warmup: ~90 tiny matmuls (N=64) lift the HAM clock gate to
            # 2.4 GHz (~3.4us of sustained PE activity) while input DMAs land.
            warm_in = small.tile([P, 64], BF16, name="warm_in", tag="warm_in")
            ones = small.tile([P, 1], BF16, name="ones", tag="ones")
            nc.vector.memset(warm_in[:], 0.0)
            nc.vector.memset(ones[:], 1.0)
            wps = warm_ps.tile([1, 64], FP32, name="wps", tag="wps")
            for _ in range(90):
                nc.tensor.matmul(wps[:], ones[:], warm_in[:], start=True, stop=True)

            # persistent SBUF arrays
            qT = [persist.tile([P, T], BF16, name=f"qT{e}", tag=f"qT{e}") for e in range(N_ET)]
            kT = [persist.tile([P, T], BF16, name=f"kT{e}", tag=f"kT{e}") for e in range(N_ET)]
            v = [persist.tile([P, C], BF16, name=f"v{t}", tag=f"v{t}") for t in range(N_TT)]
            maskT = small.tile([P, P], FP32, name="maskT", tag="maskT")
            nc.sync.dma_start(maskT[:], maskT_d[:])

            # ---------------- phase 1: projections ----------------
            with (
                tc.tile_pool(name="ph1", bufs=1) as ph1,
                tc.tile_pool(name="ph1ps", bufs=7, space="PSUM") as ph1ps,
            ):
                xT = [ph1.tile([P, T], BF16, name=f"xT{c}", tag=f"xT{c}") for c in range(n_ct)]
                wqk = [ph1.tile([P, 2 * C], BF16, name=f"wqk{c}", tag=f"wqk{c}") for c in range(n_ct)]
                wv = [ph1.tile([P, C], BF16, name=f"wv{c}", tag=f"wv{c}") for c in range(n_ct)]
                # qk inputs first (needed first), wv after
                for c in range(n_ct):
                    nc.sync.dma_start(xT[c][:], xT_d[c * P : (c + 1) * P, :])
                    nc.sync.dma_start(wqk[c][:], wqk_d[c * P : (c + 1) * P, :])
                for c in range(n_ct):
                    nc.sync.dma_start(wv[c][:], wv_d[c * P : (c + 1) * P, :])

                # qT/kT: psum[e-tile, t-chunk] = sum_c w_qk[c, e].T @ xT[c, t]
                # c is the middle loop so lhsT stays loaded across 4 matmuls.
                for e in range(2 * N_ET):
                    dst = qT[e] if e < N_ET else kT[e - N_ET]
                    pss = [
                        ph1ps.tile([P, TCH], FP32, name="qkps", tag="qkps")
                        for _ in range(T // TCH)
                    ]
                    for c in range(n_ct):
                        for tc_i in range(T // TCH):
                            nc.tensor.matmul(
                                pss[tc_i][:],
                                wqk[c][:, e * P : (e + 1) * P],
                                xT[c][:, tc_i * TCH : (tc_i + 1) * TCH],
                                start=(c == 0),
                                stop=(c == n_ct - 1),
                            )
                    for tc_i in range(T // TCH):
                        dst_ap = dst[:, tc_i * TCH : (tc_i + 1) * TCH]
                        if (e * 4 + tc_i) % 2 == 0:
                            nc.vector.tensor_copy(dst_ap, pss[tc_i][:])
                        else:
                            nc.scalar.copy(dst_ap, pss[tc_i][:])

                # v: psum[t-tile, c'-chunk] = sum_c xT[c, t].T @ w_v[c, c']
                for t in range(N_TT):
                    pss = [
                        ph1ps.tile([P, TCH], FP32, name="vps", tag="qkps")
                        for _ in range(C // TCH)
                    ]
                    for c in range(n_ct):
                        for cc in range(C // TCH):
                            nc.tensor.matmul(
                                pss[cc][:],
                                xT[c][:, t * P : (t + 1) * P],
                                wv[c][:, cc * TCH : (cc + 1) * TCH],
                                start=(c == 0),
                                stop=(c == n_ct - 1),
                            )
                    for cc in range(C // TCH):
                        dst_ap = v[t][:, cc * TCH : (cc + 1) * TCH]
                        if (t * 2 + cc) % 2 == 0:
                            nc.vector.tensor_copy(dst_ap, pss[cc][:])
                        else:
                            nc.scalar.copy(dst_ap, pss[cc][:])

            # ---------------- phase 2: attention + proj ----------------
            with (
                tc.tile_pool(name="ph2", bufs=1) as ph2,
                tc.tile_pool(name="pt_pool", bufs=18) as pt_pool,
                tc.tile_pool(name="ot_pool", bufs=3) as ot_pool,
                tc.tile_pool(name="stage", bufs=3) as stage,
                tc.tile_pool(name="st_ps", bufs=2, space="PSUM") as st_ps,
                tc.tile_pool(name="sums_ps", bufs=1, space="PSUM") as sums_ps,
                tc.tile_pool(name="ot_ps", bufs=2, space="PSUM") as ot_ps,
                tc.tile_pool(name="pr_ps", bufs=2, space="PSUM") as pr_ps,
            ):
                wp = [ph2.tile([P, C], BF16, name=f"wp{c}", tag=f"wp{c}") for c in range(N_ET)]
                for c in range(N_ET):
                    nc.sync.dma_start(wp[c][:], wp_d[c * P : (c + 1) * P, :])

                def emit_proj(t0, ot_sb, rt):
                    """proj for the supertile starting at t0, scaled by 1/sums."""
                    for k in range(TCH // P):  # t-tile within supertile
                        prs = [
                            pr_ps.tile([P, TCH], FP32, name="pr", tag="pr")
                            for _ in range(C // TCH)
                        ]
                        for g in range(N_ET):  # lhsT fixed across d-chunks
                            for dch in range(C // TCH):
                                nc.tensor.matmul(
                                    prs[dch][:],
                                    ot_sb[g][:, k * P : (k + 1) * P],
                                    wp[g][:, dch * TCH : (dch + 1) * TCH],
                                    start=(g == 0),
                                    stop=(g == N_ET - 1),
                                )
                        for dch in range(C // TCH):
                            osb_out = stage.tile([P, TCH], FP32, name="osb_out", tag="osb_out")
                            nc.scalar.activation(
                                osb_out[:],
                                prs[dch][:],
                                mybir.ActivationFunctionType.Copy,
                                scale=rt[:, k : k + 1],
                            )
                            nc.sync.dma_start(
                                out_d[
                                    t0 + k * P : t0 + (k + 1) * P,
                                    dch * TCH : (dch + 1) * TCH,
                                ],
                                osb_out[:],
                            )

                pending = None  # (t0, ot_sb, rt) of the previous supertile
                for i in range(N_SUP):  # supertile: t in [i*TCH, (i+1)*TCH)
                    t0 = i * TCH
                    n_st = 4 * i + 4  # causal s-tiles
                    ptiles = []
                    # --- ST + exp per s-tile ---
                    for j in range(n_st):
                        off = max(0, j - 4 * i) * P  # first valid t column
                        st = st_ps.tile([P, TCH], FP32, name="st", tag="st")
                        for e in range(N_ET):
                            nc.tensor.matmul(
                                st[:, off:TCH],
                                kT[e][:, j * P : (j + 1) * P],
                                qT[e][:, t0 + off : t0 + TCH],
                                start=(e == 0),
                                stop=(e == N_ET - 1),
                            )
                        if j >= 4 * i:  # diagonal block: strict-upper (s>t) mask
                            nc.vector.tensor_add(
                                st[:, off : off + P], st[:, off : off + P], maskT[:]
                            )
                        pt = pt_pool.tile([P, TCH], BF16, name="pt", tag="pt")
                        if off:
                            nc.gpsimd.memset(pt[:, :off], 0.0)
                        nc.scalar.activation(
                            pt[:, off:TCH],
                            st[:, off:TCH],
                            mybir.ActivationFunctionType.Exp,
                            scale=SCALE,
                        )
                        ptiles.append(pt)

                    # --- row sums via ones-matmul (acc over s-tiles) ---
                    sums = sums_ps.tile([1, TCH], FP32, name="sums", tag="sums")
                    for j in range(n_st):
                        nc.tensor.matmul(
                            sums[:],
                            ones[:],
                            ptiles[j][:],
                            start=(j == 0),
                            stop=(j == n_st - 1),
                        )
                    srow = stage.tile([1, TCH], FP32, name="srow", tag="srow")
                    nc.vector.tensor_copy(srow[:], sums[:])
                    nc.sync.dma_start(scr_d[i : i + 1, :], srow[:])
                    rt0 = stage.tile([P, N_SUP], FP32, name="rt0", tag="rt0")
                    nc.sync.dma_start(
                        rt0[:], scr_d[i : i + 1, :].rearrange("p (f q) -> (p q) f", q=P)
                    )
                    rt = stage.tile([P, N_SUP], FP32, name="rt", tag="rt")
                    nc.vector.reciprocal(rt[:], rt0[:])

                    # --- previous supertile's proj (hides the recip roundtrip) ---
                    if pending is not None:
                        emit_proj(*pending)

                    # --- OT[c'-tile, t-chunk] = sum_s v[s,c'].T @ P~T[s,t] ---
                    ot_sb = []
                    for g in range(N_ET):
                        ot = ot_ps.tile([P, TCH], FP32, name="ot", tag="ot")
                        for j in range(n_st):
                            nc.tensor.matmul(
                                ot[:],
                                v[j][:, g * P : (g + 1) * P],
                                ptiles[j][:],
                                start=(j == 0),
                                stop=(j == n_st - 1),
                            )
                        osb = ot_pool.tile([P, TCH], BF16, name="osb", tag=f"osb{g % 3}")
                        nc.vector.tensor_copy(osb[:], ot[:])
                        ot_sb.append(osb)

                    pending = (t0, ot_sb, rt)

                emit_proj(*pending)

    nc.compile()
    return nc


def kernel(x, w_attn, b_attn, w_proj, b_proj):
    x = np.asarray(x, dtype=np.float32)
    w_attn = np.asarray(w_attn, dtype=np.float32)
    b_attn = np.asarray(b_attn, dtype=np.float32)
    w_proj = np.asarray(w_proj, dtype=np.float32)
    b_proj = np.asarray(b_proj, dtype=np.float32)
    assert x.shape == (B, T, C)

    aug = bool(np.any(b_attn != 0.0))
    n_ct = C // P + (1 if aug else 0)
    if n_ct not in _cache:
        _cache[n_ct] = _build(n_ct)
    nc = _cache[n_ct]

    bf = ml_dtypes.bfloat16
    if aug:
        wqk = np.zeros((n_ct * P, 2 * C), dtype=bf)
        wqk[:C] = w_attn[:, : 2 * C].astype(bf)
        wqk[C] = b_attn[: 2 * C].astype(bf)
        wv = np.zeros((n_ct * P, C), dtype=bf)
        wv[:C] = w_attn[:, 2 * C :].astype(bf)
        wv[C] = b_attn[2 * C :].astype(bf)
    else:
        wqk = np.ascontiguousarray(w_attn[:, : 2 * C]).astype(bf)
        wv = np.ascontiguousarray(w_attn[:, 2 * C :]).astype(bf)
    wp = w_proj.astype(bf)

    # strict upper triangle (s > t) additive mask for transposed [s, t] blocks
    maskT = np.where(
        np.arange(P)[:, None] > np.arange(P)[None, :], np.float32(NEG), np.float32(0.0)
    ).astype(np.float32)

    in_maps = []
    for b in range(B):
        xT = np.ascontiguousarray(x[b].T).astype(bf)
        if aug:
            xTa = np.zeros((n_ct * P, T), dtype=bf)
            xTa[:C] = xT
            xTa[C] = bf(1.0)
            xT = xTa
        in_maps.append({"xT": xT, "wqk": wqk, "wv": wv, "wp": wp, "maskT": maskT})

    global _last_in_maps
    _last_in_maps = in_maps
    res = run_bass_kernel_spmd(nc, in_maps, core_ids=list(range(8)))
    out = np.stack([res.results[b]["out"] for b in range(B)]).astype(np.float32)
    if np.any(b_proj != 0.0):
        out = out + b_proj[None, None, :]
    return out


if __name__ == "__main__":
    rng = np.random.default_rng(0)
    x = rng.standard_normal((B, T, C), dtype=np.float32)
    w_attn = rng.standard_normal((C, 3 * C), dtype=np.float32) / np.sqrt(C)
    b_attn = np.zeros(3 * C, dtype=np.float32)
    w_proj = rng.standard_normal((C, C), dtype=np.float32) / np.sqrt(C)
    b_proj = np.zeros(C, dtype=np.float32)
    out = kernel(x, w_attn, b_attn, w_proj, b_proj)
    print(out.shape, out.dtype)


# revision 8
# speedup vs baseline: 1.2391x; 1.0262x over previous
"""Causal self-attention kernel for Trainium2 (8 NeuronCores, data-parallel).

Problem: B=8, T=2048, C=1024 single-head causal attention:
    qkv = x @ w_attn + b_attn ; q,k,v = split(qkv)
    attn = softmax(q @ k.T / sqrt(C) + causal_mask)
    out  = (attn @ v) @ w_proj + b_proj

Sharding: pure data parallel — one batch element per core, weights replicated,
no collectives.

Per-core algorithm (all matmuls bf16 operands, fp32 PSUM accumulate):
  host: xT = x[b].T cast bf16 (so the contraction dim is the partition dim
        everywhere on device; no on-device transposes needed anywhere).
  ph1:  qT[e,t], kT[e,s]  <- matmul(lhsT=w_qk[c,e-tile], rhs=xT[c,t])   [e,t] layout
        v[t,c']           <- matmul(lhsT=xT[c,t-tile],  rhs=w_v[c,c'])  natural layout
  ph2:  per 512-wide t-chunk ("supertile"), per 128-wide s-tile (causal only):
        ST[s,t]  <- matmul(lhsT=kT[e,s-tile], rhs=qT[e,t-chunk])  (8 e-tiles acc)
        P~T[s,t] <- exp(ST/sqrt(C) + mask)   (no max-subtract; logits are O(1))
        sums[t]  <- matmul(lhsT=ones[s,1], rhs=P~T)  (acc over s-tiles)
        OT[c',t] <- matmul(lhsT=v[s-tile,c'-tile], rhs=P~T[s-tile,t-chunk])
        out[t,d] <- matmul(lhsT=OT[c',t-tile], rhs=w_proj[c',d]) * (1/sums[t])
  The 1/sums normalization is folded into the final PSUM->SBUF copy as a
  per-partition activation scale (everything between exp and out is linear).
  The proj stage runs one supertile behind (software pipeline) so the
  sums->reciprocal DMA roundtrip never stalls the PE.

Inner loops are ordered so the matmul stationary operand is reused across
consecutive matmuls (amortizes LDWEIGHTS), and a dummy-matmul warmup stream at
t=0 lifts the PE HAM clock gate to 2.4 GHz while the first DMAs land.

b_attn is folded in by augmenting x with a ones column (padded to a full
128-partition tile) only when it is nonzero; b_proj is added on the host.
"""

import sys

if "/opt/trn_rl_repo" not in sys.path:
    sys.path.insert(0, "/opt/trn_rl_repo")

import numpy as np
import ml_dtypes

import concourse.bacc as bacc
import concourse.mybir as mybir
import concourse.tile as tile
from concourse.bass_utils import run_bass_kernel_spmd

B, T, C = 8, 2048, 1024
P = 128  # partitions
TCH = 512  # t-chunk (moving free dim)
N_TT = T // P  # 16 t-tiles
N_SUP = T // TCH  # 4 supertiles
N_ET = C // P  # 8 e-tiles (q/k feature dim)
SCALE = 1.0 / float(np.sqrt(np.float32(C)))
NEG = -10000000000.0

BF16 = mybir.dt.bfloat16
FP32 = mybir.dt.float32

_cache = {}


def _build(n_ct):
    """Build the SPMD Bass program. n_ct = number of 128-wide c-tiles of the
    (possibly ones-augmented) input feature dim."""
    nc = bacc.Bacc("TRN2", target_bir_lowering=False, debug=False, num_devices=8)

    xT_d = nc.dram_tensor("xT", [n_ct * P, T], BF16, kind="ExternalInput").ap()
    wqk_d = nc.dram_tensor("wqk", [n_ct * P, 2 * C], BF16, kind="ExternalInput").ap()
    wv_d = nc.dram_tensor("wv", [n_ct * P, C], BF16, kind="ExternalInput").ap()
    wp_d = nc.dram_tensor("wp", [C, C], BF16, kind="ExternalInput").ap()
    maskT_d = nc.dram_tensor("maskT", [P, P], FP32, kind="ExternalInput").ap()
    out_d = nc.dram_tensor("out", [T, C], FP32, kind="ExternalOutput").ap()
    scr_d = nc.dram_tensor("scr", [N_SUP, TCH], FP32, kind="ExternalOutput").ap()

    with tile.TileContext(nc) as tc:
        with (
            tc.tile_pool(name="persist", bufs=1) as persist,
            tc.tile_pool(name="small", bufs=1) as small,
        ):
            # PE warmup: ~120 tiny matmuls (N=64) lift the HAM clock gate to
            # 2.4 GHz (~3.4us of sustained PE activity) while input DMAs land.
            warm_in = small.tile([P, 64], BF16, name="warm_in", tag="warm_in")
            ones = small.tile([P, 1], BF16, name="ones", tag="ones")
            nc.vector.memset(warm_in[:], 0.0)
            nc.vector.memset(ones[:], 1.0)
            with tc.tile_pool(name="warm_ps", bufs=1, space="PSUM") as warm_ps:
                wps = warm_ps.tile([1, 64], FP32, name="wps", tag="wps")
                for _ in range(120):
                    nc.tensor.matmul(wps[:], ones[:], warm_in[:], start=True, stop=True)

            # persistent SBUF arrays
            qT = [persist.tile([P, T], BF16, name=f"qT{e}", tag=f"qT{e}") for e in range(N_ET)]
            kT = [persist.tile([P, T], BF16, name=f"kT{e}", tag=f"kT{e}") for e in range(N_ET)]
            v = [persist.tile([P, C], BF16, name=f"v{t}", tag=f"v{t}") for t in range(N_TT)]
            maskT = small.tile([P, P], FP32, name="maskT", tag="maskT")
            nc.sync.dma_start(maskT[:], maskT_d[:])

            # ---------------- phase 1: projections ----------------
            with (
                tc.tile_pool(name="ph1", bufs=1) as ph1,
                tc.tile_pool(name="ph1ps", bufs=8, space="PSUM") as ph1ps,
            ):
                xT = [ph1.tile([P, T], BF16, name=f"xT{c}", tag=f"xT{c}") for c in range(n_ct)]
                wqk = [ph1.tile([P, 2 * C], BF16, name=f"wqk{c}", tag=f"wqk{c}") for c in range(n_ct)]
                wv = [ph1.tile([P, C], BF16, name=f"wv{c}", tag=f"wv{c}") for c in range(n_ct)]
                # qk inputs first (needed first); wv loads are issued mid-qk
                # (see below) so they don't compete during the DMA ramp.
                for c in range(n_ct):
                    nc.sync.dma_start(xT[c][:], xT_d[c * P : (c + 1) * P, :])
                    nc.sync.dma_start(wqk[c][:], wqk_d[c * P : (c + 1) * P, :])

                # qT/kT: psum[e-tile, t-chunk] = sum_c w_qk[c, e].T @ xT[c, t]
                # c is the middle loop so lhsT stays loaded across 4 matmuls.
                for e in range(2 * N_ET):
                    if e == 8:
                        for c in range(n_ct):
                            nc.sync.dma_start(wv[c][:], wv_d[c * P : (c + 1) * P, :])
                    dst = qT[e] if e < N_ET else kT[e - N_ET]
                    pss = [
                        ph1ps.tile([P, TCH], FP32, name="qkps", tag="qkps")
                        for _ in range(T // TCH)
                    ]
                    for c in range(n_ct):
                        for tc_i in range(T // TCH):
                            nc.tensor.matmul(
                                pss[tc_i][:],
                                wqk[c][:, e * P : (e + 1) * P],
                                xT[c][:, tc_i * TCH : (tc_i + 1) * TCH],
                                start=(c == 0),
                                stop=(c == n_ct - 1),
                            )
                    for tc_i in range(T // TCH):
                        dst_ap = dst[:, tc_i * TCH : (tc_i + 1) * TCH]
                        if (e * 4 + tc_i) % 2 == 0:
                            nc.vector.tensor_copy(dst_ap, pss[tc_i][:])
                        else:
                            nc.scalar.copy(dst_ap, pss[tc_i][:])

                # v: psum[t-tile, c'-chunk] = sum_c xT[c, t].T @ w_v[c, c']
                for t in range(N_TT):
                    pss = [
                        ph1ps.tile([P, TCH], FP32, name="vps", tag="qkps")
                        for _ in range(C // TCH)
                    ]
                    for c in range(n_ct):
                        for cc in range(C // TCH):
                            nc.tensor.matmul(
                                pss[cc][:],
                                xT[c][:, t * P : (t + 1) * P],
                                wv[c][:, cc * TCH : (cc + 1) * TCH],
                                start=(c == 0),
                                stop=(c == n_ct - 1),
                            )
                    for cc in range(C // TCH):
                        dst_ap = v[t][:, cc * TCH : (cc + 1) * TCH]
                        if (t * 2 + cc) % 2 == 0:
                            nc.vector.tensor_copy(dst_ap, pss[cc][:])
                        else:
                            nc.scalar.copy(dst_ap, pss[cc][:])

            # ---------------- phase 2: attention + proj ----------------
            with (
                tc.tile_pool(name="ph2", bufs=1) as ph2,
                tc.tile_pool(name="pt_pool", bufs=18) as pt_pool,
                tc.tile_pool(name="ot_pool", bufs=3) as ot_pool,
                tc.tile_pool(name="stage", bufs=3) as stage,
                tc.tile_pool(name="st_ps", bufs=2, space="PSUM") as st_ps,
                tc.tile_pool(name="sums_ps", bufs=1, space="PSUM") as sums_ps,
                tc.tile_pool(name="ot_ps", bufs=2, space="PSUM") as ot_ps,
                tc.tile_pool(name="pr_ps", bufs=2, space="PSUM") as pr_ps,
            ):
                wp = [ph2.tile([P, C], BF16, name=f"wp{c}", tag=f"wp{c}") for c in range(N_ET)]
                for c in range(N_ET):
                    nc.sync.dma_start(wp[c][:], wp_d[c * P : (c + 1) * P, :])

                def emit_proj(t0, ot_sb, rt):
                    """proj for the supertile starting at t0, scaled by 1/sums."""
                    for k in range(TCH // P):  # t-tile within supertile
                        prs = [
                            pr_ps.tile([P, TCH], FP32, name="pr", tag="pr")
                            for _ in range(C // TCH)
                        ]
                        for g in range(N_ET):  # lhsT fixed across d-chunks
                            for dch in range(C // TCH):
                                nc.tensor.matmul(
                                    prs[dch][:],
                                    ot_sb[g][:, k * P : (k + 1) * P],
                                    wp[g][:, dch * TCH : (dch + 1) * TCH],
                                    start=(g == 0),
                                    stop=(g == N_ET - 1),
                                )
                        for dch in range(C // TCH):
                            osb_out = stage.tile([P, TCH], FP32, name="osb_out", tag="osb_out")
                            if dch % 2 == 0:
                                nc.scalar.activation(
                                    osb_out[:],
                                    prs[dch][:],
                                    mybir.ActivationFunctionType.Copy,
                                    scale=rt[:, k : k + 1],
                                )
                            else:
                                nc.vector.tensor_scalar_mul(
                                    osb_out[:], prs[dch][:], rt[:, k : k + 1]
                                )
                            nc.sync.dma_start(
                                out_d[
                                    t0 + k * P : t0 + (k + 1) * P,
                                    dch * TCH : (dch + 1) * TCH,
                                ],
                                osb_out[:],
                            )

                pending = None  # (t0, ot_sb, rt) of the previous supertile
                for i in range(N_SUP):  # supertile: t in [i*TCH, (i+1)*TCH)
                    t0 = i * TCH
                    n_st = 4 * i + 4  # causal s-tiles
                    ptiles = []
                    # --- ST + exp per s-tile ---
                    for j in range(n_st):
                        off = max(0, j - 4 * i) * P  # first valid t column
                        st = st_ps.tile([P, TCH], FP32, name="st", tag="st")
                        for e in range(N_ET):
                            nc.tensor.matmul(
                                st[:, off:TCH],
                                kT[e][:, j * P : (j + 1) * P],
                                qT[e][:, t0 + off : t0 + TCH],
                                start=(e == 0),
                                stop=(e == N_ET - 1),
                            )
                        if j >= 4 * i:  # diagonal block: strict-upper (s>t) mask
                            nc.vector.tensor_add(
                                st[:, off : off + P], st[:, off : off + P], maskT[:]
                            )
                        pt = pt_pool.tile([P, TCH], BF16, name="pt", tag="pt")
                        nc.scalar.activation(
                            pt[:, off:TCH],
                            st[:, off:TCH],
                            mybir.ActivationFunctionType.Exp,
                            scale=SCALE,
                        )
                        ptiles.append((pt, off))

                    # --- row sums via ones-matmul (acc over s-tiles) ---
                    # j=0 always has off=0, so the first (start=True) matmul
                    # covers the full width; later partial-width matmuls
                    # accumulate into their column subrange only.
                    sums = sums_ps.tile([1, TCH], FP32, name="sums", tag="sums")
                    for j in range(n_st):
                        pt, off = ptiles[j]
                        nc.tensor.matmul(
                            sums[:, off:TCH],
                            ones[:],
                            pt[:, off:TCH],
                            start=(j == 0),
                            stop=(j == n_st - 1),
                        )
                    srow = stage.tile([1, TCH], FP32, name="srow", tag="srow")
                    nc.vector.tensor_copy(srow[:], sums[:])
                    nc.sync.dma_start(scr_d[i : i + 1, :], srow[:])
                    rt0 = stage.tile([P, N_SUP], FP32, name="rt0", tag="rt0")
                    nc.sync.dma_start(
                        rt0[:], scr_d[i : i + 1, :].rearrange("p (f q) -> (p q) f", q=P)
                    )
                    rt = stage.tile([P, N_SUP], FP32, name="rt", tag="rt")
                    nc.vector.reciprocal(rt[:], rt0[:])

                    # --- previous supertile's proj (hides the recip roundtrip) ---
                    if pending is not None:
                        emit_proj(*pending)

                    # --- OT[c'-tile, t-chunk] = sum_s v[s,c'].T @ P~T[s,t] ---
                    ot_sb = []
                    for g in range(N_ET):
                        ot = ot_ps.tile([P, TCH], FP32, name="ot", tag="ot")
                        for j in range(n_st):
                            pt, off = ptiles[j]
                            nc.tensor.matmul(
                                ot[:, off:TCH],
                                v[j][:, g * P : (g + 1) * P],
                                pt[:, off:TCH],
                                start=(j == 0),
                                stop=(j == n_st - 1),
                            )
                        osb = ot_pool.tile([P, TCH], BF16, name="osb", tag=f"osb{g % 3}")
                        nc.vector.tensor_copy(osb[:], ot[:])
                        ot_sb.append(osb)

                    pending = (t0, ot_sb, rt)

                emit_proj(*pending)

    nc.compile()
    return nc


def kernel(x, w_attn, b_attn, w_proj, b_proj):
    x = np.asarray(x, dtype=np.float32)
    w_attn = np.asarray(w_attn, dtype=np.float32)
    b_attn = np.asarray(b_attn, dtype=np.float32)
    w_proj = np.asarray(w_proj, dtype=np.float32)
    b_proj = np.asarray(b_proj, dtype=np.float32)
    assert x.shape == (B, T, C)

    aug = bool(np.any(b_attn != 0.0))
    n_ct = C // P + (1 if aug else 0)
    if n_ct not in _cache:
        _cache[n_ct] = _build(n_ct)
    nc = _cache[n_ct]

    bf = ml_dtypes.bfloat16
    if aug:
        wqk = np.zeros((n_ct * P, 2 * C), dtype=bf)
        wqk[:C] = w_attn[:, : 2 * C].astype(bf)
        wqk[C] = b_attn[: 2 * C].astype(bf)
        wv = np.zeros((n_ct * P, C), dtype=bf)
        wv[:C] = w_attn[:, 2 * C :].astype(bf)
        wv[C] = b_attn[2 * C :].astype(bf)
    else:
        wqk = np.ascontiguousarray(w_attn[:, : 2 * C]).astype(bf)
        wv = np.ascontiguousarray(w_attn[:, 2 * C :]).astype(bf)
    wp = w_proj.astype(bf)

    # strict upper triangle (s > t) additive mask for transposed [s, t] blocks
    maskT = np.where(
        np.arange(P)[:, None] > np.arange(P)[None, :], np.float32(NEG), np.float32(0.0)
    ).astype(np.float32)

    in_maps = []
    for b in range(B):
        xT = np.ascontiguousarray(x[b].T).astype(bf)
        if aug:
            xTa = np.zeros((n_ct * P, T), dtype=bf)
            xTa[:C] = xT
            xTa[C] = bf(1.0)
            xT = xTa
        in_maps.append({"xT": xT, "wqk": wqk, "wv": wv, "wp": wp, "maskT": maskT})

    global _last_in_maps
    _last_in_maps = in_maps
    res = run_bass_kernel_spmd(nc, in_maps, core_ids=list(range(8)))
    out = np.stack([res.results[b]["out"] for b in range(B)]).astype(np.float32)
    if np.any(b_proj != 0.0):
        out = out + b_proj[None, None, :]
    return out


if __name__ == "__main__":
    rng = np.random.default_rng(0)
    x = rng.standard_normal((B, T, C), dtype=np.float32)
    w_attn = rng.standard_normal((C, 3 * C), dtype=np.float32) / np.sqrt(C)
    b_attn = np.zeros(3 * C, dtype=np.float32)
    w_proj = rng.standard_normal((C, C), dtype=np.float32) / np.sqrt(C)
    b_proj = np.zeros(C, dtype=np.float32)
    out = kernel(x, w_attn, b_attn, w_proj, b_proj)
    print(out.shape, out.dtype)
